# revision 1
# baseline (speedup 1.0000x reference)
"""DeepSet (phi -> segment_sum -> rho) Bass kernel for 8 trn2 NeuronCores.

Strategy (data-parallel over segments, per sharding hint):
  - 16384 segments split into 8 cores x 2048 segments (segment-aligned row
    ranges, computed on host via searchsorted on the sorted segment_ids).
  - Per core, segments are processed in 16 windows of 128 segments. Rows of
    each window are padded (host-side) to T tiles of 128 rows so the SPMD
    program is static and identical across cores.
  - phi runs with features on partitions: h = W.T @ xT (W stationary).
    The phi third layer is commuted past the pooling (it is linear):
    pooled3 = W3.T @ segsum(h2) + b3 * counts.
  - Pooling is a matmul: pooledT[65, 128segs] += h2T_aug[128rows, 65].T @
    onehot[128rows, 128segs], PSUM-accumulated over the window's tiles.
    h2T comes from a PE transpose; onehot is built on DVE from the
    (host-prepared) per-row window-relative segment ids vs an iota.
    The 65th column of h2T_aug is ones -> row 64 of pooledT = counts.
  - rho (+ the commuted phi L3 with bias-via-counts) runs per window on
    [*, 128] tiles; output written as [4, 2048] per core, host transposes.
"""

import sys

import numpy as np

sys.path.insert(0, "/opt/trn_rl_repo")

import concourse.bass as bass  # noqa: E402
import concourse.mybir as mybir  # noqa: E402
import concourse.tile as tile  # noqa: E402
from concourse import bacc  # noqa: E402
from concourse.bass_utils import run_bass_kernel_spmd  # noqa: E402
from concourse.masks import make_identity  # noqa: E402

F32 = mybir.dt.float32
I32 = mybir.dt.int32
AF = mybir.ActivationFunctionType

NUM_SEGMENTS = 16384
N_CORES = 8
SEG_PER_CORE = NUM_SEGMENTS // N_CORES  # 2048
WIN_SEGS = 128
N_WIN = SEG_PER_CORE // WIN_SEGS  # 16
STATE_DIM = 8
HID = 64
OUT_DIM = 4

_BUILD_CACHE: dict[int, object] = {}


def _build_program(T: int):
    """Build + compile the per-core Bass program for T 128-row tiles/window."""
    if T in _BUILD_CACHE:
        return _BUILD_CACHE[T]

    PW = T * 128  # padded rows per window

    nc = bacc.Bacc("TRN2", target_bir_lowering=False, debug=False, num_devices=N_CORES)

    xT_d = nc.declare_dram_parameter("xT", [STATE_DIM, N_WIN * PW], F32, isOutput=False)
    idr_d = nc.declare_dram_parameter("idr", [128, N_WIN * T], F32, isOutput=False)
    w1_d = nc.declare_dram_parameter("w1", [STATE_DIM, HID], F32, isOutput=False)
    w2_d = nc.declare_dram_parameter("w2", [HID, HID], F32, isOutput=False)
    w3a_d = nc.declare_dram_parameter("w3a", [HID + 1, HID], F32, isOutput=False)
    rw1_d = nc.declare_dram_parameter("rw1", [HID, HID], F32, isOutput=False)
    rw2_d = nc.declare_dram_parameter("rw2", [HID, HID], F32, isOutput=False)
    rw3_d = nc.declare_dram_parameter("rw3", [HID, OUT_DIM], F32, isOutput=False)
    pb1_d = nc.declare_dram_parameter("pb1", [HID, 1], F32, isOutput=False)
    pb2_d = nc.declare_dram_parameter("pb2", [HID, 1], F32, isOutput=False)
    rb1_d = nc.declare_dram_parameter("rb1", [HID, 1], F32, isOutput=False)
    rb2_d = nc.declare_dram_parameter("rb2", [HID, 1], F32, isOutput=False)
    rb3_d = nc.declare_dram_parameter("rb3", [OUT_DIM, 1], F32, isOutput=False)
    out_d = nc.declare_dram_parameter("out", [OUT_DIM, SEG_PER_CORE], F32, isOutput=True)

    with tile.TileContext(nc) as tc:
        with (
            tc.tile_pool(name="const", bufs=1) as cpool,
            tc.tile_pool(name="xwin", bufs=2) as xpool,
            tc.tile_pool(name="work", bufs=3) as wpool,
            tc.tile_pool(name="mmps", bufs=3, space="PSUM") as mmps,
            tc.tile_pool(name="poolps", bufs=2, space="PSUM") as poolps,
            tc.tile_pool(name="tailps", bufs=2, space="PSUM") as tailps,
        ):
            w1 = cpool.tile([STATE_DIM, HID], F32)
            nc.sync.dma_start(out=w1[:], in_=w1_d[:])
            w2 = cpool.tile([HID, HID], F32)
            nc.sync.dma_start(out=w2[:], in_=w2_d[:])
            w3a = cpool.tile([HID + 1, HID], F32)
            nc.sync.dma_start(out=w3a[:], in_=w3a_d[:])
            rw1 = cpool.tile([HID, HID], F32)
            nc.sync.dma_start(out=rw1[:], in_=rw1_d[:])
            rw2 = cpool.tile([HID, HID], F32)
            nc.sync.dma_start(out=rw2[:], in_=rw2_d[:])
            rw3 = cpool.tile([HID, OUT_DIM], F32)
            nc.sync.dma_start(out=rw3[:], in_=rw3_d[:])
            pb1 = cpool.tile([HID, 1], F32)
            nc.sync.dma_start(out=pb1[:], in_=pb1_d[:])
            pb2 = cpool.tile([HID, 1], F32)
            nc.sync.dma_start(out=pb2[:], in_=pb2_d[:])
            rb1 = cpool.tile([HID, 1], F32)
            nc.sync.dma_start(out=rb1[:], in_=rb1_d[:])
            rb2 = cpool.tile([HID, 1], F32)
            nc.sync.dma_start(out=rb2[:], in_=rb2_d[:])
            rb3 = cpool.tile([OUT_DIM, 1], F32)
            nc.sync.dma_start(out=rb3[:], in_=rb3_d[:])

            idr = cpool.tile([128, N_WIN * T], F32)
            nc.sync.dma_start(out=idr[:], in_=idr_d[:])

            ident = cpool.tile([HID, HID], F32)
            make_identity(nc, ident[:])
            iota_i = cpool.tile([128, 128], I32)
            nc.gpsimd.iota(iota_i[:], pattern=[[1, 128]], base=0, channel_multiplier=0)
            iota_f = cpool.tile([128, 128], F32)
            nc.vector.tensor_copy(out=iota_f[:], in_=iota_i[:])

            for w in range(N_WIN):
                xw = xpool.tile([STATE_DIM, PW], F32, tag="xw")
                nc.sync.dma_start(out=xw[:], in_=xT_d[:, w * PW : (w + 1) * PW])

                pooled_ps = poolps.tile([HID + 1, WIN_SEGS], F32, tag="pool")

                for t in range(T):
                    cols = slice(t * 128, (t + 1) * 128)
                    h1_ps = mmps.tile([HID, 128], F32, tag="mm")
                    nc.tensor.matmul(
                        out=h1_ps[:], lhsT=w1[:], rhs=xw[:, cols], start=True, stop=True
                    )
                    h1_sb = wpool.tile([HID, 128], F32, tag="h1")
                    nc.scalar.activation(
                        out=h1_sb[:], in_=h1_ps[:], func=AF.Relu, bias=pb1[:]
                    )
                    h2_ps = mmps.tile([HID, 128], F32, tag="mm")
                    nc.tensor.matmul(
                        out=h2_ps[:], lhsT=w2[:], rhs=h1_sb[:], start=True, stop=True
                    )
                    h2_sb = wpool.tile([HID, 128], F32, tag="h2")
                    nc.scalar.activation(
                        out=h2_sb[:], in_=h2_ps[:], func=AF.Relu, bias=pb2[:]
                    )
                    h2t_ps = mmps.tile([128, HID], F32, tag="mm")
                    nc.tensor.transpose(out=h2t_ps[:], in_=h2_sb[:], identity=ident[:])
                    h2ta = wpool.tile([128, HID + 1], F32, tag="h2ta")
                    nc.vector.tensor_copy(out=h2ta[:, :HID], in_=h2t_ps[:])
                    nc.vector.memset(h2ta[:, HID : HID + 1], 1.0)

                    onehot = wpool.tile([128, 128], F32, tag="onehot")
                    col = w * T + t
                    nc.vector.tensor_tensor(
                        out=onehot[:],
                        in0=idr[:, col : col + 1].to_broadcast([128, 128]),
                        in1=iota_f[:],
                        op=mybir.AluOpType.is_equal,
                    )
                    nc.tensor.matmul(
                        out=pooled_ps[:],
                        lhsT=h2ta[:],
                        rhs=onehot[:],
                        start=(t == 0),
                        stop=(t == T - 1),
                    )

                pooled_sb = wpool.tile([HID + 1, WIN_SEGS], F32, tag="pooled")
                nc.vector.tensor_copy(out=pooled_sb[:], in_=pooled_ps[:])

                p3_ps = tailps.tile([HID, WIN_SEGS], F32, tag="tail")
                nc.tensor.matmul(
                    out=p3_ps[:], lhsT=w3a[:], rhs=pooled_sb[:], start=True, stop=True
                )
                p3_sb = wpool.tile([HID, WIN_SEGS], F32, tag="p3")
                nc.scalar.copy(out=p3_sb[:], in_=p3_ps[:])

                r1_ps = tailps.tile([HID, WIN_SEGS], F32, tag="tail")
                nc.tensor.matmul(
                    out=r1_ps[:], lhsT=rw1[:], rhs=p3_sb[:], start=True, stop=True
                )
                r1_sb = wpool.tile([HID, WIN_SEGS], F32, tag="r1")
                nc.scalar.activation(out=r1_sb[:], in_=r1_ps[:], func=AF.Relu, bias=rb1[:])

                r2_ps = tailps.tile([HID, WIN_SEGS], F32, tag="tail")
                nc.tensor.matmul(
                    out=r2_ps[:], lhsT=rw2[:], rhs=r1_sb[:], start=True, stop=True
                )
                r2_sb = wpool.tile([HID, WIN_SEGS], F32, tag="r2")
                nc.scalar.activation(out=r2_sb[:], in_=r2_ps[:], func=AF.Relu, bias=rb2[:])

                r3_ps = tailps.tile([OUT_DIM, WIN_SEGS], F32, tag="tail")
                nc.tensor.matmul(
                    out=r3_ps[:], lhsT=rw3[:], rhs=r2_sb[:], start=True, stop=True
                )
                out_sb = wpool.tile([OUT_DIM, WIN_SEGS], F32, tag="outw")
                nc.vector.tensor_scalar(
                    out=out_sb[:],
                    in0=r3_ps[:],
                    scalar1=rb3[:],
                    scalar2=None,
                    op0=mybir.AluOpType.add,
                )
                nc.sync.dma_start(
                    out=out_d[:, w * WIN_SEGS : (w + 1) * WIN_SEGS], in_=out_sb[:]
                )

    nc.compile()
    _BUILD_CACHE[T] = nc
    return nc


def _prep_inputs(neighbors: np.ndarray, segment_ids: np.ndarray):
    """Shard rows by 128-segment windows, pad each window to T 128-row tiles."""
    x = np.asarray(neighbors, dtype=np.float32)
    ids = np.asarray(segment_ids, dtype=np.int64)
    n_gwin = NUM_SEGMENTS // WIN_SEGS  # 128 global windows
    edges = np.searchsorted(ids, np.arange(0, NUM_SEGMENTS + 1, WIN_SEGS))
    wcnt = np.diff(edges)
    T = max(1, int(np.ceil(wcnt.max() / 128)))
    PW = T * 128

    xT = np.zeros((N_CORES, STATE_DIM, N_WIN * PW), dtype=np.float32)
    idr = np.full((N_CORES, 128, N_WIN * T), -1.0, dtype=np.float32)
    for g in range(n_gwin):
        c, wl = divmod(g, N_WIN)
        r0, r1 = int(edges[g]), int(edges[g + 1])
        n = r1 - r0
        if n == 0:
            continue
        base = wl * PW
        xT[c, :, base : base + n] = x[r0:r1].T
        rel = np.full(PW, -1.0, dtype=np.float32)
        rel[:n] = (ids[r0:r1] - g * WIN_SEGS).astype(np.float32)
        idr[c, :, wl * T : (wl + 1) * T] = rel.reshape(T, 128).T
    return xT, idr, T


def kernel(
    neighbors,
    segment_ids,
    phi_W1,
    phi_b1,
    phi_W2,
    phi_b2,
    phi_W3,
    phi_b3,
    rho_W1,
    rho_b1,
    rho_W2,
    rho_b2,
    rho_W3,
    rho_b3,
):
    xT, idr, T = _prep_inputs(neighbors, segment_ids)
    nc = _build_program(T)

    f = lambda a: np.ascontiguousarray(np.asarray(a, dtype=np.float32))
    col = lambda a: f(a).reshape(-1, 1)
    shared = {
        "w1": f(phi_W1),
        "w2": f(phi_W2),
        "w3a": np.vstack([f(phi_W3), f(phi_b3).reshape(1, -1)]),
        "rw1": f(rho_W1),
        "rw2": f(rho_W2),
        "rw3": f(rho_W3),
        "pb1": col(phi_b1),
        "pb2": col(phi_b2),
        "rb1": col(rho_b1),
        "rb2": col(rho_b2),
        "rb3": col(rho_b3),
    }
    in_maps = [
        {"xT": xT[c], "idr": idr[c], **shared} for c in range(N_CORES)
    ]
    res = run_bass_kernel_spmd(nc, in_maps, core_ids=list(range(N_CORES)))
    out = np.concatenate(
        [res.results[c]["out"].T for c in range(N_CORES)], axis=0
    ).astype(np.float32)
    return out


# revision 3
# speedup vs baseline: 695.8657x; 695.8657x over previous
"""DeepSet (phi -> segment_sum -> rho) Bass kernel for 8 trn2 NeuronCores.

Strategy (data-parallel over segments, per sharding hint):
  - 16384 segments split into 8 cores x 2048 segments (segment-aligned row
    ranges, computed on host via searchsorted on the sorted segment_ids).
  - Per core, segments are processed in 16 windows of 128 segments. Rows of
    each window are padded (host-side) to T tiles of 128 rows so the SPMD
    program is static and identical across cores.
  - phi runs with features on partitions: h = W.T @ xT (W stationary).
    The phi third layer is commuted past the pooling (it is linear):
    pooled3 = W3.T @ segsum(h2) + b3 * counts.
  - Pooling is a matmul: pooledT[65, 128segs] += h2T_aug[128rows, 65].T @
    onehot[128rows, 128segs], PSUM-accumulated over the window's tiles.
    h2T comes from a PE transpose; onehot is built on DVE from the
    (host-prepared) per-row window-relative segment ids vs an iota.
    The 65th column of h2T_aug is ones -> row 64 of pooledT = counts.
  - rho (+ the commuted phi L3 with bias-via-counts) runs per window on
    [*, 128] tiles; output written as [4, 2048] per core, host transposes.
"""

import sys

import numpy as np

sys.path.insert(0, "/opt/trn_rl_repo")

import concourse.bass as bass  # noqa: E402
import concourse.mybir as mybir  # noqa: E402
import concourse.tile as tile  # noqa: E402
from concourse import bacc  # noqa: E402
from concourse.bass_utils import run_bass_kernel_spmd  # noqa: E402
from concourse.masks import make_identity  # noqa: E402

F32 = mybir.dt.float32
I32 = mybir.dt.int32
AF = mybir.ActivationFunctionType

NUM_SEGMENTS = 16384
N_CORES = 8
SEG_PER_CORE = NUM_SEGMENTS // N_CORES  # 2048
WIN_SEGS = 128
N_WIN = SEG_PER_CORE // WIN_SEGS  # 16
STATE_DIM = 8
HID = 64
OUT_DIM = 4

_BUILD_CACHE: dict[tuple[int, int], object] = {}


def _build_program(T: int, reps: int = 1):
    """Build + compile the per-core Bass program for T 128-row tiles/window.

    reps > 1 repeats the whole compute (same inputs/outputs) for
    differential wall-clock timing; results are identical.
    """
    if (T, reps) in _BUILD_CACHE:
        return _BUILD_CACHE[(T, reps)]

    PW = T * 128  # padded rows per window

    nc = bacc.Bacc("TRN2", target_bir_lowering=False, debug=False, num_devices=N_CORES)

    xT_d = nc.declare_dram_parameter("xT", [STATE_DIM, N_WIN * PW], F32, isOutput=False)
    idr_d = nc.declare_dram_parameter("idr", [128, N_WIN * T], F32, isOutput=False)
    w1_d = nc.declare_dram_parameter("w1", [STATE_DIM, HID], F32, isOutput=False)
    w2_d = nc.declare_dram_parameter("w2", [HID, HID], F32, isOutput=False)
    w3a_d = nc.declare_dram_parameter("w3a", [HID + 1, HID], F32, isOutput=False)
    rw1_d = nc.declare_dram_parameter("rw1", [HID, HID], F32, isOutput=False)
    rw2_d = nc.declare_dram_parameter("rw2", [HID, HID], F32, isOutput=False)
    rw3_d = nc.declare_dram_parameter("rw3", [HID, OUT_DIM], F32, isOutput=False)
    pb1_d = nc.declare_dram_parameter("pb1", [HID, 1], F32, isOutput=False)
    pb2_d = nc.declare_dram_parameter("pb2", [HID, 1], F32, isOutput=False)
    rb1_d = nc.declare_dram_parameter("rb1", [HID, 1], F32, isOutput=False)
    rb2_d = nc.declare_dram_parameter("rb2", [HID, 1], F32, isOutput=False)
    rb3_d = nc.declare_dram_parameter("rb3", [OUT_DIM, 1], F32, isOutput=False)
    out_d = nc.declare_dram_parameter("out", [OUT_DIM, SEG_PER_CORE], F32, isOutput=True)

    with tile.TileContext(nc) as tc:
        with (
            tc.tile_pool(name="const", bufs=1) as cpool,
            tc.tile_pool(name="xwin", bufs=2) as xpool,
            tc.tile_pool(name="work", bufs=3) as wpool,
            tc.tile_pool(name="mmps", bufs=3, space="PSUM") as mmps,
            tc.tile_pool(name="poolps", bufs=2, space="PSUM") as poolps,
            tc.tile_pool(name="tailps", bufs=2, space="PSUM") as tailps,
        ):
            w1 = cpool.tile([STATE_DIM, HID], F32)
            nc.sync.dma_start(out=w1[:], in_=w1_d[:])
            w2 = cpool.tile([HID, HID], F32)
            nc.sync.dma_start(out=w2[:], in_=w2_d[:])
            w3a = cpool.tile([HID + 1, HID], F32)
            nc.sync.dma_start(out=w3a[:], in_=w3a_d[:])
            rw1 = cpool.tile([HID, HID], F32)
            nc.sync.dma_start(out=rw1[:], in_=rw1_d[:])
            rw2 = cpool.tile([HID, HID], F32)
            nc.sync.dma_start(out=rw2[:], in_=rw2_d[:])
            rw3 = cpool.tile([HID, OUT_DIM], F32)
            nc.sync.dma_start(out=rw3[:], in_=rw3_d[:])
            pb1 = cpool.tile([HID, 1], F32)
            nc.sync.dma_start(out=pb1[:], in_=pb1_d[:])
            pb2 = cpool.tile([HID, 1], F32)
            nc.sync.dma_start(out=pb2[:], in_=pb2_d[:])
            rb1 = cpool.tile([HID, 1], F32)
            nc.sync.dma_start(out=rb1[:], in_=rb1_d[:])
            rb2 = cpool.tile([HID, 1], F32)
            nc.sync.dma_start(out=rb2[:], in_=rb2_d[:])
            rb3 = cpool.tile([OUT_DIM, 1], F32)
            nc.sync.dma_start(out=rb3[:], in_=rb3_d[:])

            idr = cpool.tile([128, N_WIN * T], F32)
            nc.sync.dma_start(out=idr[:], in_=idr_d[:])

            ident = cpool.tile([HID, HID], F32)
            make_identity(nc, ident[:])
            iota_i = cpool.tile([128, 128], I32)
            nc.gpsimd.iota(iota_i[:], pattern=[[1, 128]], base=0, channel_multiplier=0)
            iota_f = cpool.tile([128, 128], F32)
            nc.vector.tensor_copy(out=iota_f[:], in_=iota_i[:])

            for _rep in range(reps):
             for w in range(N_WIN):
                xw = xpool.tile([STATE_DIM, PW], F32, tag="xw")
                nc.sync.dma_start(out=xw[:], in_=xT_d[:, w * PW : (w + 1) * PW])

                pooled_ps = poolps.tile([HID + 1, WIN_SEGS], F32, tag="pool")

                for t in range(T):
                    cols = slice(t * 128, (t + 1) * 128)
                    h1_ps = mmps.tile([HID, 128], F32, tag="mm")
                    nc.tensor.matmul(
                        out=h1_ps[:], lhsT=w1[:], rhs=xw[:, cols], start=True, stop=True
                    )
                    h1_sb = wpool.tile([HID, 128], F32, tag="h1")
                    nc.scalar.activation(
                        out=h1_sb[:], in_=h1_ps[:], func=AF.Relu, bias=pb1[:]
                    )
                    h2_ps = mmps.tile([HID, 128], F32, tag="mm")
                    nc.tensor.matmul(
                        out=h2_ps[:], lhsT=w2[:], rhs=h1_sb[:], start=True, stop=True
                    )
                    h2_sb = wpool.tile([HID, 128], F32, tag="h2")
                    nc.scalar.activation(
                        out=h2_sb[:], in_=h2_ps[:], func=AF.Relu, bias=pb2[:]
                    )
                    h2t_ps = mmps.tile([128, HID], F32, tag="mm")
                    nc.tensor.transpose(out=h2t_ps[:], in_=h2_sb[:], identity=ident[:])
                    h2ta = wpool.tile([128, HID + 1], F32, tag="h2ta")
                    nc.vector.tensor_copy(out=h2ta[:, :HID], in_=h2t_ps[:])
                    nc.vector.memset(h2ta[:, HID : HID + 1], 1.0)

                    onehot = wpool.tile([128, 128], F32, tag="onehot")
                    col = w * T + t
                    nc.vector.tensor_tensor(
                        out=onehot[:],
                        in0=idr[:, col : col + 1].to_broadcast([128, 128]),
                        in1=iota_f[:],
                        op=mybir.AluOpType.is_equal,
                    )
                    nc.tensor.matmul(
                        out=pooled_ps[:],
                        lhsT=h2ta[:],
                        rhs=onehot[:],
                        start=(t == 0),
                        stop=(t == T - 1),
                    )

                pooled_sb = wpool.tile([HID + 1, WIN_SEGS], F32, tag="pooled")
                nc.vector.tensor_copy(out=pooled_sb[:], in_=pooled_ps[:])

                p3_ps = tailps.tile([HID, WIN_SEGS], F32, tag="tail")
                nc.tensor.matmul(
                    out=p3_ps[:], lhsT=w3a[:], rhs=pooled_sb[:], start=True, stop=True
                )
                p3_sb = wpool.tile([HID, WIN_SEGS], F32, tag="p3")
                nc.scalar.copy(out=p3_sb[:], in_=p3_ps[:])

                r1_ps = tailps.tile([HID, WIN_SEGS], F32, tag="tail")
                nc.tensor.matmul(
                    out=r1_ps[:], lhsT=rw1[:], rhs=p3_sb[:], start=True, stop=True
                )
                r1_sb = wpool.tile([HID, WIN_SEGS], F32, tag="r1")
                nc.scalar.activation(out=r1_sb[:], in_=r1_ps[:], func=AF.Relu, bias=rb1[:])

                r2_ps = tailps.tile([HID, WIN_SEGS], F32, tag="tail")
                nc.tensor.matmul(
                    out=r2_ps[:], lhsT=rw2[:], rhs=r1_sb[:], start=True, stop=True
                )
                r2_sb = wpool.tile([HID, WIN_SEGS], F32, tag="r2")
                nc.scalar.activation(out=r2_sb[:], in_=r2_ps[:], func=AF.Relu, bias=rb2[:])

                r3_ps = tailps.tile([OUT_DIM, WIN_SEGS], F32, tag="tail")
                nc.tensor.matmul(
                    out=r3_ps[:], lhsT=rw3[:], rhs=r2_sb[:], start=True, stop=True
                )
                out_sb = wpool.tile([OUT_DIM, WIN_SEGS], F32, tag="outw")
                nc.vector.tensor_scalar(
                    out=out_sb[:],
                    in0=r3_ps[:],
                    scalar1=rb3[:],
                    scalar2=None,
                    op0=mybir.AluOpType.add,
                )
                nc.sync.dma_start(
                    out=out_d[:, w * WIN_SEGS : (w + 1) * WIN_SEGS], in_=out_sb[:]
                )

    nc.compile()
    _BUILD_CACHE[T] = nc
    return nc


def _prep_inputs(neighbors: np.ndarray, segment_ids: np.ndarray):
    """Shard rows by 128-segment windows, pad each window to T 128-row tiles."""
    x = np.asarray(neighbors, dtype=np.float32)
    ids = np.asarray(segment_ids, dtype=np.int64)
    n_gwin = NUM_SEGMENTS // WIN_SEGS  # 128 global windows
    edges = np.searchsorted(ids, np.arange(0, NUM_SEGMENTS + 1, WIN_SEGS))
    wcnt = np.diff(edges)
    T = max(1, int(np.ceil(wcnt.max() / 128)))
    PW = T * 128

    xT = np.zeros((N_CORES, STATE_DIM, N_WIN * PW), dtype=np.float32)
    idr = np.full((N_CORES, 128, N_WIN * T), -1.0, dtype=np.float32)
    for g in range(n_gwin):
        c, wl = divmod(g, N_WIN)
        r0, r1 = int(edges[g]), int(edges[g + 1])
        n = r1 - r0
        if n == 0:
            continue
        base = wl * PW
        xT[c, :, base : base + n] = x[r0:r1].T
        rel = np.full(PW, -1.0, dtype=np.float32)
        rel[:n] = (ids[r0:r1] - g * WIN_SEGS).astype(np.float32)
        idr[c, :, wl * T : (wl + 1) * T] = rel.reshape(T, 128).T
    return xT, idr, T


def kernel(
    neighbors,
    segment_ids,
    phi_W1,
    phi_b1,
    phi_W2,
    phi_b2,
    phi_W3,
    phi_b3,
    rho_W1,
    rho_b1,
    rho_W2,
    rho_b2,
    rho_W3,
    rho_b3,
):
    xT, idr, T = _prep_inputs(neighbors, segment_ids)
    nc = _build_program(T)

    f = lambda a: np.ascontiguousarray(np.asarray(a, dtype=np.float32))
    col = lambda a: f(a).reshape(-1, 1)
    shared = {
        "w1": f(phi_W1),
        "w2": f(phi_W2),
        "w3a": np.vstack([f(phi_W3), f(phi_b3).reshape(1, -1)]),
        "rw1": f(rho_W1),
        "rw2": f(rho_W2),
        "rw3": f(rho_W3),
        "pb1": col(phi_b1),
        "pb2": col(phi_b2),
        "rb1": col(rho_b1),
        "rb2": col(rho_b2),
        "rb3": col(rho_b3),
    }
    in_maps = [
        {"xT": xT[c], "idr": idr[c], **shared} for c in range(N_CORES)
    ]
    res = run_bass_kernel_spmd(nc, in_maps, core_ids=list(range(N_CORES)))
    out = np.concatenate(
        [res.results[c]["out"].T for c in range(N_CORES)], axis=0
    ).astype(np.float32)
    return out


# revision 7
# speedup vs baseline: 2456.4581x; 3.5301x over previous
"""DeepSet (phi -> segment_sum -> rho) Bass kernel for 8 trn2 NeuronCores.

Strategy (data-parallel over segments, per sharding hint):
  - 16384 segments -> 8 cores x 2048 segments (segment-aligned row ranges via
    host-side searchsorted on the sorted segment_ids).
  - Per core: 16 windows of 128 segments; each window's rows padded host-side
    to T 128-row tiles (T global) so the SPMD program is static.
  - phi L1 with weights stationary: h1 = W1a.T @ xT -> [65, rows]; feature 64
    is a constant-one row (W1a col 64 = 0, bias 1) which provides phi L2's
    bias through the contraction.
  - phi L2 with h1 stationary (two concurrent 64-column strips of the PE
    array): h2 = h1a.T @ W2a -> [128 rows, 64 feat] - the orientation pooling
    needs, with no transpose.
  - phi L3 is linear, so it commutes past the pooling:
    pooled3 = W3.T @ segsum(h2) + b3 * counts (counts computed on host).
  - Pooling is a matmul: pooled[64segs.. wait, pooled[64feat?] - see code:
    pooledT[feat, seg] += h2t[128rows, 64feat].T @ onehot[128rows, 128segs]
    PSUM-accumulated over the window's tiles. onehot is one batched DVE
    is_equal per 4 tiles against a repeated iota.
  - rho per window on [*, 128] tiles; output [4, 2048] per core, host
    transposes and concatenates.
"""

import sys

import numpy as np

sys.path.insert(0, "/opt/trn_rl_repo")

import concourse.bass as bass  # noqa: E402
import concourse.mybir as mybir  # noqa: E402
import concourse.tile as tile  # noqa: E402
from concourse import bacc  # noqa: E402
from concourse.bass_utils import run_bass_kernel_spmd  # noqa: E402

F32 = mybir.dt.float32
I32 = mybir.dt.int32
AF = mybir.ActivationFunctionType

NUM_SEGMENTS = 16384
N_CORES = 8
SEG_PER_CORE = NUM_SEGMENTS // N_CORES  # 2048
WIN_SEGS = 128
N_WIN = SEG_PER_CORE // WIN_SEGS  # 16
STATE_DIM = 8
HID = 64
OUT_DIM = 4
GRP = 4  # tiles per op-batch group (512 rows)

_BUILD_CACHE: dict[tuple[int, int], object] = {}


def _build_program(T: int, reps: int = 1):
    """Build + compile the per-core program; T = 128-row tiles per window
    (multiple of GRP). reps>1 repeats the compute for differential timing."""
    if (T, reps) in _BUILD_CACHE:
        return _BUILD_CACHE[(T, reps)]
    assert T % GRP == 0
    PW = T * 128
    NG = T // GRP  # groups per window

    nc = bacc.Bacc("TRN2", target_bir_lowering=False, debug=False, num_devices=N_CORES)

    xT_d = nc.declare_dram_parameter("xT", [STATE_DIM, N_WIN * PW], F32, isOutput=False)
    idr_d = nc.declare_dram_parameter("idr", [128, N_WIN * T], F32, isOutput=False)
    cnt_d = nc.declare_dram_parameter("cnt", [N_WIN, WIN_SEGS], F32, isOutput=False)
    w1a_d = nc.declare_dram_parameter("w1a", [STATE_DIM, HID + 1], F32, isOutput=False)
    w2a_d = nc.declare_dram_parameter("w2a", [HID + 1, HID], F32, isOutput=False)
    w3a_d = nc.declare_dram_parameter("w3a", [HID + 1, HID], F32, isOutput=False)
    rw1_d = nc.declare_dram_parameter("rw1", [HID, HID], F32, isOutput=False)
    rw2_d = nc.declare_dram_parameter("rw2", [HID, HID], F32, isOutput=False)
    rw3_d = nc.declare_dram_parameter("rw3", [HID, OUT_DIM], F32, isOutput=False)
    pb1a_d = nc.declare_dram_parameter("pb1a", [HID + 1, 1], F32, isOutput=False)
    rb1_d = nc.declare_dram_parameter("rb1", [HID, 1], F32, isOutput=False)
    rb2_d = nc.declare_dram_parameter("rb2", [HID, 1], F32, isOutput=False)
    rb3_d = nc.declare_dram_parameter("rb3", [OUT_DIM, 1], F32, isOutput=False)
    out_d = nc.declare_dram_parameter("out", [OUT_DIM, SEG_PER_CORE], F32, isOutput=True)

    with tile.TileContext(nc) as tc:
        with (
            tc.tile_pool(name="const", bufs=1) as cpool,
            tc.tile_pool(name="xwin", bufs=2) as xpool,
            tc.tile_pool(name="work", bufs=3) as wpool,
            tc.tile_pool(name="h1ps", bufs=2, space="PSUM") as h1ps,
            tc.tile_pool(name="h2ps", bufs=2, space="PSUM") as h2ps,
            tc.tile_pool(name="poolps", bufs=2, space="PSUM") as poolps,
            tc.tile_pool(name="tailps", bufs=2, space="PSUM") as tailps,
        ):
            def cload(name, shape, dram):
                t = cpool.tile(shape, F32, tag=name)
                nc.sync.dma_start(out=t[:], in_=dram[:])
                return t

            w1a = cload("w1a", [STATE_DIM, HID + 1], w1a_d)
            w2a = cload("w2a", [HID + 1, HID], w2a_d)
            w3a = cload("w3a", [HID + 1, HID], w3a_d)
            rw1 = cload("rw1", [HID, HID], rw1_d)
            rw2 = cload("rw2", [HID, HID], rw2_d)
            rw3 = cload("rw3", [HID, OUT_DIM], rw3_d)
            pb1a = cload("pb1a", [HID + 1, 1], pb1a_d)
            rb1 = cload("rb1", [HID, 1], rb1_d)
            rb2 = cload("rb2", [HID, 1], rb2_d)
            rb3 = cload("rb3", [OUT_DIM, 1], rb3_d)
            idr = cload("idr", [128, N_WIN * T], idr_d)

            iota_i = cpool.tile([128, GRP * 128], I32)
            nc.gpsimd.iota(
                iota_i[:], pattern=[[0, GRP], [1, 128]], base=0, channel_multiplier=0
            )
            iota4 = cpool.tile([128, GRP * 128], F32)
            nc.vector.tensor_copy(out=iota4[:], in_=iota_i[:])

            for _rep in range(reps):
             for w in range(N_WIN):
                xw = xpool.tile([STATE_DIM, PW], F32, tag="xw")
                nc.sync.dma_start(out=xw[:], in_=xT_d[:, w * PW : (w + 1) * PW])

                pooled_ps = poolps.tile([HID, WIN_SEGS], F32, tag="pool")

                for g in range(NG):
                    gcols = slice(g * GRP * 128, (g + 1) * GRP * 128)
                    h1_ps = h1ps.tile([HID + 1, GRP * 128], F32, tag="h1")
                    nc.tensor.matmul(
                        out=h1_ps[:], lhsT=w1a[:], rhs=xw[:, gcols],
                        start=True, stop=True,
                    )
                    h1a = wpool.tile([HID + 1, GRP * 128], F32, tag="h1a")
                    nc.scalar.activation(
                        out=h1a[:], in_=h1_ps[:], func=AF.Relu, bias=pb1a[:]
                    )

                    h2_ps = h2ps.tile([128, GRP * HID], F32, tag="h2")
                    for t in range(GRP):
                        for s in range(2):
                            nc.tensor.matmul(
                                out=h2_ps[s * 64 : s * 64 + 64, t * HID : (t + 1) * HID],
                                lhsT=h1a[:, t * 128 + s * 64 : t * 128 + s * 64 + 64],
                                rhs=w2a[:],
                                start=True,
                                stop=True,
                                tile_position=(0, s * 64),
                            )
                    h2t = wpool.tile([128, GRP * HID], F32, tag="h2t")
                    nc.vector.tensor_scalar(
                        out=h2t[:], in0=h2_ps[:], scalar1=0.0, scalar2=None,
                        op0=mybir.AluOpType.max,
                    )

                    onehot = wpool.tile([128, GRP * 128], F32, tag="onehot")
                    c0 = w * T + g * GRP
                    nc.vector.tensor_tensor(
                        out=onehot[:].rearrange("p (a b) -> p a b", b=128),
                        in0=idr[:, c0 : c0 + GRP].to_broadcast([128, GRP, 128]),
                        in1=iota4[:].rearrange("p (a b) -> p a b", b=128),
                        op=mybir.AluOpType.is_equal,
                    )
                    for t in range(GRP):
                        nc.tensor.matmul(
                            out=pooled_ps[:],
                            lhsT=h2t[:, t * HID : (t + 1) * HID],
                            rhs=onehot[:, t * 128 : (t + 1) * 128],
                            start=(g == 0 and t == 0),
                            stop=(g == NG - 1 and t == GRP - 1),
                        )

                pooled_sb = wpool.tile([HID + 1, WIN_SEGS], F32, tag="pooled")
                nc.vector.tensor_copy(out=pooled_sb[:HID, :], in_=pooled_ps[:])
                nc.sync.dma_start(
                    out=pooled_sb[HID : HID + 1, :], in_=cnt_d[w : w + 1, :]
                )

                p3_ps = tailps.tile([HID, WIN_SEGS], F32, tag="tail")
                nc.tensor.matmul(
                    out=p3_ps[:], lhsT=w3a[:], rhs=pooled_sb[:], start=True, stop=True
                )
                p3_sb = wpool.tile([HID, WIN_SEGS], F32, tag="p3")
                nc.scalar.copy(out=p3_sb[:], in_=p3_ps[:])

                r1_ps = tailps.tile([HID, WIN_SEGS], F32, tag="tail")
                nc.tensor.matmul(
                    out=r1_ps[:], lhsT=rw1[:], rhs=p3_sb[:], start=True, stop=True
                )
                r1_sb = wpool.tile([HID, WIN_SEGS], F32, tag="r1")
                nc.scalar.activation(out=r1_sb[:], in_=r1_ps[:], func=AF.Relu, bias=rb1[:])

                r2_ps = tailps.tile([HID, WIN_SEGS], F32, tag="tail")
                nc.tensor.matmul(
                    out=r2_ps[:], lhsT=rw2[:], rhs=r1_sb[:], start=True, stop=True
                )
                r2_sb = wpool.tile([HID, WIN_SEGS], F32, tag="r2")
                nc.scalar.activation(out=r2_sb[:], in_=r2_ps[:], func=AF.Relu, bias=rb2[:])

                r3_ps = tailps.tile([OUT_DIM, WIN_SEGS], F32, tag="tail")
                nc.tensor.matmul(
                    out=r3_ps[:], lhsT=rw3[:], rhs=r2_sb[:], start=True, stop=True
                )
                out_sb = wpool.tile([OUT_DIM, WIN_SEGS], F32, tag="outw")
                nc.vector.tensor_scalar(
                    out=out_sb[:], in0=r3_ps[:], scalar1=rb3[:], scalar2=None,
                    op0=mybir.AluOpType.add,
                )
                nc.sync.dma_start(
                    out=out_d[:, w * WIN_SEGS : (w + 1) * WIN_SEGS], in_=out_sb[:]
                )

    nc.compile()
    _BUILD_CACHE[(T, reps)] = nc
    return nc


def _prep_inputs(neighbors: np.ndarray, segment_ids: np.ndarray):
    """Shard rows by 128-segment windows; pad each window to T 128-row tiles."""
    x = np.asarray(neighbors, dtype=np.float32)
    ids = np.asarray(segment_ids, dtype=np.int64)
    n_gwin = NUM_SEGMENTS // WIN_SEGS  # 128 global windows
    edges = np.searchsorted(ids, np.arange(0, NUM_SEGMENTS + 1, WIN_SEGS))
    wcnt = np.diff(edges)
    T = max(GRP, GRP * int(np.ceil(wcnt.max() / (128 * GRP))))
    PW = T * 128

    xT = np.zeros((N_CORES, STATE_DIM, N_WIN * PW), dtype=np.float32)
    idr = np.full((N_CORES, 128, N_WIN * T), -1.0, dtype=np.float32)
    counts = np.bincount(ids, minlength=NUM_SEGMENTS).astype(np.float32)
    cnt = counts.reshape(N_CORES, N_WIN, WIN_SEGS)
    for g in range(n_gwin):
        c, wl = divmod(g, N_WIN)
        r0, r1 = int(edges[g]), int(edges[g + 1])
        n = r1 - r0
        if n == 0:
            continue
        base = wl * PW
        xT[c, :, base : base + n] = x[r0:r1].T
        rel = np.full(PW, -1.0, dtype=np.float32)
        rel[:n] = (ids[r0:r1] - g * WIN_SEGS).astype(np.float32)
        idr[c, :, wl * T : (wl + 1) * T] = rel.reshape(T, 128).T
    return xT, idr, cnt, T


def prep_maps(inputs: dict):
    """Host-side marshalling: returns (T, in_maps per core)."""
    xT, idr, cnt, T = _prep_inputs(inputs["neighbors"], inputs["segment_ids"])
    f = lambda a: np.ascontiguousarray(np.asarray(a, dtype=np.float32))
    col = lambda a: f(a).reshape(-1, 1)
    w1a = np.concatenate([f(inputs["phi_W1"]), np.zeros((STATE_DIM, 1), np.float32)], 1)
    pb1a = np.concatenate([col(inputs["phi_b1"]), np.ones((1, 1), np.float32)], 0)
    w2a = np.vstack([f(inputs["phi_W2"]), f(inputs["phi_b2"]).reshape(1, -1)])
    w3a = np.vstack([f(inputs["phi_W3"]), f(inputs["phi_b3"]).reshape(1, -1)])
    shared = {
        "w1a": w1a,
        "w2a": w2a,
        "w3a": w3a,
        "rw1": f(inputs["rho_W1"]),
        "rw2": f(inputs["rho_W2"]),
        "rw3": f(inputs["rho_W3"]),
        "pb1a": pb1a,
        "rb1": col(inputs["rho_b1"]),
        "rb2": col(inputs["rho_b2"]),
        "rb3": col(inputs["rho_b3"]),
    }
    in_maps = [
        {"xT": xT[c], "idr": idr[c], "cnt": cnt[c], **shared} for c in range(N_CORES)
    ]
    return T, in_maps


def kernel(**inputs):
    T, in_maps = prep_maps(inputs)
    nc = _build_program(T)
    res = run_bass_kernel_spmd(nc, in_maps, core_ids=list(range(N_CORES)))
    out = np.concatenate(
        [res.results[c]["out"].T for c in range(N_CORES)], axis=0
    ).astype(np.float32)
    return out
